# revision 51
# baseline (speedup 1.0000x reference)
"""Trainium2 Bass kernel for causal multi-head attention with ALiBi.

Computes, for x:[B,S,D]:
    qkv = x @ W_packed.T + b_packed ; q,k,v = split(qkv)
    heads -> scores = q k^T / sqrt(hd) + alibi_causal_bias
    out = softmax(scores) v -> merge heads -> out @ W_out.T + b_out

Sharding (8 cores): core c handles batch c//4 and heads {k, k+4, k+8, k+12}
(k = c%4), one head per "slot". Slot block-schedules are head-independent
(sized for the largest ALiBi window in the slot), so one SPMD program runs
on all 8 cores; only the data (weight slices, bias tables) differs.
Host sums the 4 out-projection partials per batch and adds
b_out + W_out @ b_v (the v-bias term commutes through attention).

ALiBi sparsity: head h attends effectively only ~24/slope_h positions back.
Slots keep only the causal k-blocks within that window (KEEP blocks).

Softmax without row-max: scores are O(+-6); exp is recentred per q-group by
a constant that cancels in normalization. In the transposed layout
scoresT[k,q] the ALiBi bias slope*(k - C_g) is per-partition so it rides
the Exp activation bias. Row sums come from a ones-row appended to v;
normalization divides by that row.

v3 pipeline:
- Lead-in computes only q/k columns 0-511 and v blocks 0-3 (m-outer, DMA
  overlapped); attention group 0 starts right after the x DMA lands. The
  remaining QKV work and the out-projection (one group behind) are woven
  between attention steps as PE filler bursts.
- Attention runs two lanes (slots A,B) of k-block PAIRS: each step does
  2 score MMs per lane into one 2-bank PSUM tile and ONE wide Exp over
  both blocks. The two blocks' ALiBi biases differ by slope*128; that
  offset is folded into pre-scaled copies of v (lanes 4-7 of v_t hold
  v*exp(-slope*128)), so one activation bias column serves the pair.
  Slots C and D follow as a single lane (C pairs, then D entries with
  both KEEP blocks merged the same way).
- A/B score MMs contract 64 rows on complementary partition halves and
  are emitted adjacently, so the PE runs them concurrently (row tiling).
"""

import os
import sys

import numpy as np

for _p in ("/opt/trn_rl_repo",):
    if os.path.isdir(_p) and _p not in sys.path:
        sys.path.append(_p)

import concourse.bacc as bacc
import concourse.bass as bass
import concourse.tile as tile
from concourse import mybir
from concourse.bass_utils import run_bass_kernel_spmd

B, S, D, H, HD = 2, 2048, 1024, 16, 64
NBLK = S // 128          # 16 k/q blocks
NCORES = 8

F32 = mybir.dt.float32
F32R = mybir.dt.float32r
BF16 = mybir.dt.bfloat16

# Slots A..D: per-core heads [12+k, 8+k, 4+k, k].  KEEP = causal k-blocks
# kept per q-block (window ~24/slope_h, slot max).  W = q-group width.
SLOT_KEEP = (17, 17, 6, 2)
SLOT_W = (512, 512, 512, 128)
SLOT_OFF0 = (128, 128, 128, 64)
SLOT_TABW = tuple(k + 3 if w == 512 else k for k, w in zip(SLOT_KEEP, SLOT_W))
SLOT_TABOFF = tuple(int(np.cumsum((0,) + SLOT_TABW)[i]) for i in range(4))
TABW = int(sum(SLOT_TABW))

# One Exp per k-block pair (True) vs per k-block (False). Merging halves
# ACT instructions via the pre-scaled v lanes; measured 9us faster than
# unmerged (174.4us vs 183.6us) at rel err 1.15e-2 vs 4.0e-3 — both well
# inside the 2e-2 gate, so the faster variant wins.
MERGED_EXP = True


def _slot_schedule(s):
    """Yield (g, q0, W, [(j, lo, tabcol, isdiag), ...]) per q-group."""
    K, W, _ = SLOT_KEEP[s], SLOT_W[s], SLOT_OFF0[s]
    out = []
    if W == 512:
        for g in range(S // 512):
            jlo = max(0, 4 * g + 3 - (K - 1))
            assert jlo % 2 == 0
            blocks = []
            for j in range(jlo, 4 * g + 4):
                lo = max(0, (j - 4 * g) * 128)
                m = j - 4 * g + (K - 1)
                blocks.append((j, lo, SLOT_TABOFF[s] + m, j >= 4 * g))
            assert len(blocks) % 2 == 0
            out.append((g, g * 512, 512, blocks))
    else:
        for i in range(NBLK):
            blocks = []
            for j in range(max(0, i - (K - 1)), i + 1):
                m = j - i + (K - 1)
                blocks.append((j, 0, SLOT_TABOFF[s] + m, j == i))
            out.append((i, i * 128, 128, blocks))
    return out


def build_program():
    nc = bacc.Bacc("TRN2", target_bir_lowering=False, debug=False,
                   num_devices=NCORES)

    xT = nc.dram_tensor("xT", [D, S], BF16, kind="ExternalInput")
    wqkT = nc.dram_tensor("wqkT", [D, 512], BF16, kind="ExternalInput")
    wvT = nc.dram_tensor("wvT", [D, 256], BF16, kind="ExternalInput")
    woT = nc.dram_tensor("woT", [256, D], BF16, kind="ExternalInput")
    bqk = nc.dram_tensor("bqk", [128, 4], F32, kind="ExternalInput")
    btab = nc.dram_tensor("btab", [128, TABW], F32, kind="ExternalInput")
    onesd = nc.dram_tensor("onesd", [65, 64], F32R, kind="ExternalInput")
    vscr = nc.dram_tensor("vscr", [128, 4, 64], F32, kind="ExternalInput")
    vsco = nc.dram_tensor("vsco", [128, 4, 16], F32, kind="ExternalInput")
    out = nc.dram_tensor("out", [S, D], BF16, kind="ExternalOutput")

    sched = [_slot_schedule(s) for s in range(4)]

    with tile.TileContext(nc) as tc:
        with tc.tile_pool(name="persist", bufs=1) as pp:
            qkT = [pp.tile([128, S], BF16, tag=f"qkT{t}", name=f"qkT{t}")
                   for t in range(4)]
            # v lanes 0-3: slot v (+ones col); lanes 4-7: v*exp(-slope*128)
            v_t = pp.tile([128, 8, NBLK, 65], BF16, tag="v", name="v")
            hoT = [pp.tile([128, S], BF16, tag=f"hoT{t}", name=f"hoT{t}")
                   for t in range(2)]
            btab_sb = pp.tile([128, TABW], F32, tag="btab", name="btab")
            bqk_sb = pp.tile([128, 4], F32, tag="bqk", name="bqk")
            ones_r = pp.tile([65, 64], F32R, tag="ones_r", name="ones_r")
            vscr_sb = pp.tile([128, 4, 64], F32, tag="vscr", name="vscr")
            vsco_sb = pp.tile([128, 4, 16, 1], F32, tag="vsco", name="vsco")
            xT_sb = [pp.tile([128, S], BF16, tag=f"x{m}", name=f"x{m}")
                     for m in range(8)]
            wqk_sb = [pp.tile([128, 512], BF16, tag=f"wqk{m}", name=f"wqk{m}")
                      for m in range(8)]
            wv_sb = [pp.tile([128, 256], BF16, tag=f"wv{m}", name=f"wv{m}")
                     for m in range(8)]
            wo_sb = [pp.tile([128, D], BF16, tag=f"wo{cc}", name=f"wo{cc}")
                     for cc in range(2)]

            # DMA priority: the 4MB x stream is the critical path — split
            # it across both queues; weights follow on the sync queue.
            for m in range(8):
                q = nc.sync if m % 2 == 0 else nc.gpsimd
                q.dma_start(xT_sb[m][:], xT[m * 128:(m + 1) * 128, :])
            for m in range(8):
                nc.sync.dma_start(wqk_sb[m][:],
                                  wqkT[m * 128:(m + 1) * 128, :])
            for m in range(8):
                nc.gpsimd.dma_start(wv_sb[m][:],
                                    wvT[m * 128:(m + 1) * 128, :])
            nc.gpsimd.dma_start(btab_sb[:], btab[:])
            nc.gpsimd.dma_start(bqk_sb[:], bqk[:])
            nc.gpsimd.dma_start(ones_r[:], onesd[:])
            nc.gpsimd.dma_start(vscr_sb[:], vscr[:])
            nc.gpsimd.dma_start(vsco_sb[:], vsco[:])
            # ones col for lanes 0-3; scaled ones col for lanes 4-7
            nc.gpsimd.memset(v_t[:, 0:4, :, 64:65], 1.0)
            nc.vector.tensor_copy(v_t[:, 4:8, :, 64:65], vsco_sb[:])
            for cc in range(2):
                nc.gpsimd.dma_start(wo_sb[cc][:],
                                    woT[cc * 128:(cc + 1) * 128, :])

            # PSUM: 8 banks = sc2 x2 (2 banks each) + av x2 + op x2.
            with (
                tc.tile_pool(name="etp", bufs=8) as etp,
                tc.tile_pool(name="nrm", bufs=3) as nrm,
                tc.tile_pool(name="obp", bufs=2) as obp,
                tc.tile_pool(name="ps_sc", bufs=2, space="PSUM") as sc_ps,
                tc.tile_pool(name="ps_av", bufs=2, space="PSUM") as av_ps,
                tc.tile_pool(name="ps_op", bufs=2, space="PSUM") as op_ps,
            ):
                def qk_store(ft, q4, ps):
                    scol = slice(q4 * 512, (q4 + 1) * 512)
                    # psum*scale + bias (1/sqrt(hd) folded into q)
                    nc.vector.tensor_scalar(
                        out=qkT[ft][:, scol], in0=ps[:],
                        scalar1=(0.125 if ft < 2 else 1.0),
                        scalar2=bqk_sb[:, ft:ft + 1],
                        op0=mybir.AluOpType.mult,
                        op1=mybir.AluOpType.add,
                    )

                def v_store(sb, ps):
                    re = ps[:].rearrange("p (s c) -> p s c", s=4)
                    nc.vector.tensor_copy(v_t[:, 0:4, sb, 0:64], re)
                    nc.vector.tensor_mul(v_t[:, 4:8, sb, 0:64], re,
                                         vscr_sb[:])

                def lead_in():
                    # q/k cols 0-511 + v blocks 0-3, m-outer (DMA-overlapped)
                    qk_ps = [sc_ps.tile([128, 1024], F32, tag="sc2",
                                        name=f"li_qk{i}") for i in range(2)]
                    v_ps = ([av_ps.tile([128, 256], F32, tag="av",
                                        name=f"li_v{i}") for i in range(2)]
                            + [op_ps.tile([128, 256], F32, tag="op",
                                          name=f"li_v{i + 2}")
                               for i in range(2)])
                    for m in range(8):
                        for ft in range(4):
                            nc.tensor.matmul(
                                qk_ps[ft // 2][:, (ft % 2) * 512:
                                               (ft % 2) * 512 + 512],
                                wqk_sb[m][:, ft * 128:(ft + 1) * 128],
                                xT_sb[m][:, 0:512],
                                start=(m == 0), stop=(m == 7),
                            )
                        for sb in range(4):
                            nc.tensor.matmul(
                                v_ps[sb][:],
                                xT_sb[m][:, sb * 128:(sb + 1) * 128],
                                wv_sb[m][:],
                                start=(m == 0), stop=(m == 7),
                            )
                    # kT (ft2) and qT (ft0) first: group-0 scores need them
                    for ft in (2, 0, 3, 1):
                        qk_store(ft, 0, qk_ps[ft // 2][:, (ft % 2) * 512:
                                                       (ft % 2) * 512 + 512])
                    for sb in range(4):
                        v_store(sb, v_ps[sb])

                # ---- fillers (PE bursts woven between attention steps) ----
                # each filler is split into two half-bursts so one burst
                # never adds a full 1.7us to the attention chain; pieces
                # stay adjacent in the list (at most one open accumulation,
                # which the op-tag rotation requires)
                def qk_filler(ft, q4):
                    st = {}

                    def p1():
                        st["ps"] = op_ps.tile([128, 512], F32, tag="op",
                                              name="qkf")
                        for m in range(4):
                            nc.tensor.matmul(
                                st["ps"][:],
                                wqk_sb[m][:, ft * 128:(ft + 1) * 128],
                                xT_sb[m][:, q4 * 512:(q4 + 1) * 512],
                                start=(m == 0), stop=False,
                            )

                    def p2():
                        for m in range(4, 8):
                            nc.tensor.matmul(
                                st["ps"][:],
                                wqk_sb[m][:, ft * 128:(ft + 1) * 128],
                                xT_sb[m][:, q4 * 512:(q4 + 1) * 512],
                                start=False, stop=(m == 7),
                            )
                        qk_store(ft, q4, st["ps"])
                    return [p1, p2]

                def v_filler(sb):
                    st = {}

                    def p1():
                        st["ps"] = op_ps.tile([128, 256], F32, tag="op",
                                              name="vf")
                        for m in range(4):
                            nc.tensor.matmul(
                                st["ps"][:],
                                xT_sb[m][:, sb * 128:(sb + 1) * 128],
                                wv_sb[m][:],
                                start=(m == 0), stop=False,
                            )

                    def p2():
                        for m in range(4, 8):
                            nc.tensor.matmul(
                                st["ps"][:],
                                xT_sb[m][:, sb * 128:(sb + 1) * 128],
                                wv_sb[m][:],
                                start=False, stop=(m == 7),
                            )
                        v_store(sb, st["ps"])
                    return [p1, p2]

                def op_filler(sb):
                    st = {}

                    def half(jh):
                        ps = op_ps.tile([128, 512], F32, tag="op",
                                        name="opps")
                        for cc in range(2):
                            nc.tensor.matmul(
                                ps[:],
                                hoT[cc][:, sb * 128:(sb + 1) * 128],
                                wo_sb[cc][:, jh * 512:(jh + 1) * 512],
                                start=(cc == 0), stop=(cc == 1),
                            )
                        nc.vector.tensor_copy(
                            st["ob"][:, jh * 512:(jh + 1) * 512], ps[:])

                    def p1():
                        st["ob"] = obp.tile([128, D], BF16, tag="ob",
                                            name="ob")
                        half(0)

                    def p2():
                        half(1)
                        q = nc.sync if sb % 2 == 0 else nc.gpsimd
                        q.dma_start(out[sb * 128:(sb + 1) * 128, :],
                                    st["ob"][:])
                    return [p1, p2]

                # ---- attention lanes ----
                def vlane(s, j, isdiag_member):
                    # merged-exp pairs use the 2nd member's bias col; the
                    # 1st member (even j for A/B/C, off-diag for D) rides
                    # the pre-scaled v lane instead.
                    if not MERGED_EXP:
                        return s
                    if s == 3:
                        return 3 if isdiag_member else 7
                    return (4 + s) if (j % 2 == 0) else s

                def emit_pair_sc(u, half):
                    ln = u["ln"]
                    s = ln["s"]
                    g, q0, W, blocks = ln["ent"]
                    pr = u["pair"]
                    if half >= len(pr):
                        return
                    j, lo, tcol, isdiag = pr[half]
                    po = (s % 2) * 64
                    qT_s = qkT[s // 2][po:po + 64, :]
                    kT_s = qkT[2 + s // 2][po:po + 64, :]
                    nc.tensor.matmul(
                        u["sc"][:, half * W + lo:half * W + W],
                        kT_s[:, j * 128:(j + 1) * 128],
                        qT_s[:, q0 + lo:q0 + W],
                    )

                def emit_pair_exp(u):
                    ln = u["ln"]
                    g, q0, W, blocks = ln["ent"]
                    pr = u["pair"]
                    lo0 = pr[0][1]
                    wd = W * len(pr)
                    tcol = pr[-1][2]
                    et = etp.tile([128, 1024], BF16, tag="et", name="et")
                    if MERGED_EXP:
                        nc.scalar.activation(
                            et[:, lo0:wd], u["sc"][:, lo0:wd],
                            mybir.ActivationFunctionType.Exp,
                            bias=btab_sb[:, tcol:tcol + 1], scale=1.0,
                        )
                    else:
                        for half, (j, lo, tc, isdiag) in enumerate(pr):
                            o = half * W
                            nc.scalar.activation(
                                et[:, o + lo:o + W], u["sc"][:, o + lo:o + W],
                                mybir.ActivationFunctionType.Exp,
                                bias=btab_sb[:, tc:tc + 1], scale=1.0,
                            )
                    for half, (j, lo, _tc, isdiag) in enumerate(pr):
                        if isdiag:
                            o = half * W + lo
                            # zero k>q inside the diagonal 128x128 block
                            nc.gpsimd.affine_select(
                                out=et[:, o:o + 128],
                                in_=et[:, o:o + 128],
                                compare_op=mybir.AluOpType.is_ge,
                                fill=0.0, base=0,
                                pattern=[[1, 128]],
                                channel_multiplier=-1,
                            )
                    u["et"] = et

                def emit_pair_av(u):
                    ln = u["ln"]
                    s = ln["s"]
                    g, q0, W, blocks = ln["ent"]
                    pr = u["pair"]
                    if ln["av"] is None:
                        ln["av"] = av_ps.tile([65, 512], F32, tag="av",
                                              name="av")
                    for half, (j, lo, _tc, isdiag) in enumerate(pr):
                        bi = u["bi"] + half
                        nc.tensor.matmul(
                            ln["av"][:, lo:W],
                            v_t[:, vlane(s, j, isdiag), j, :],
                            u["et"][:, half * W + lo:half * W + W],
                            start=(bi == 0),
                            stop=(bi == len(blocks) - 1),
                        )

                def norm_part1(ln):
                    # denominator row: av psum -> SBUF (DVE, same
                    # partition), then DMA-shift to physical partition 0
                    # for the gpsimd broadcast (DVE can't shift; DMA can)
                    g, q0, W, blocks = ln["ent"]
                    lr = nrm.tile([65, 512], F32, tag="lr", name="lr")
                    nc.vector.tensor_copy(lr[64:65, :W],
                                          ln["av"][64:65, :W])
                    lr0 = nrm.tile([1, 512], F32, tag="lr0", name="lr0")
                    nc.gpsimd.dma_start(lr0[0:1, :W], lr[64:65, :W])
                    ln["lr0"] = lr0

                def norm_part2(ln):
                    s = ln["s"]
                    g, q0, W, blocks = ln["ent"]
                    po = (s % 2) * 64
                    hoT_s = hoT[s // 2]
                    av, lr0 = ln["av"], ln["lr0"]
                    # gpsimd broadcast (reads physical partition 0)
                    # replaces the PE ones-matmul: 28 fewer PE MMs and
                    # no op-tag psum churn
                    bsum = nrm.tile([64, 512], F32, tag="bsum", name="bsum")
                    nc.gpsimd.partition_broadcast(
                        bsum[:, :W], lr0[0:1, :W], channels=64)
                    binv = nrm.tile([64, 512], F32, tag="binv", name="binv")
                    nc.vector.reciprocal_approx_fast(out=binv[:, :W],
                                                     in_=bsum[:, :W])
                    if po == 0:
                        nc.vector.tensor_mul(
                            hoT_s[0:64, q0:q0 + W], av[0:64, :W],
                            binv[:, :W])
                    else:
                        # DVE lanes can't shift partitions; bounce via DMA
                        tmp = nrm.tile([64, 512], BF16, tag="hotmp",
                                       name="hotmp")
                        nc.vector.tensor_mul(tmp[:, :W], av[0:64, :W],
                                             binv[:, :W])
                        nc.gpsimd.dma_start(
                            hoT_s[64:128, q0:q0 + W], tmp[:, :W])

                def attn_group(g, fillers):
                    # phase 1: lanes A,B in k-block pairs; phase 2: C pairs
                    # then D entries, two pair-units per step, single chain.
                    def mklane(s, ent):
                        return {"s": s, "ent": ent, "av": None}

                    phase1 = [mklane(0, sched[0][g]), mklane(1, sched[1][g])]
                    phase2 = ([mklane(2, sched[2][g])]
                              + [mklane(3, sched[3][4 * g + i])
                                 for i in range(4)])
                    n1 = len(sched[0][g][3]) // 2
                    work = []
                    for st in range(n1):
                        work.append([{"ln": ln, "bi": st * 2}
                                     for ln in phase1])
                    p2 = []
                    for ln in phase2:
                        nb = len(ln["ent"][3])
                        p2.extend({"ln": ln, "bi": b}
                                  for b in range(0, nb, 2))
                    for i in range(0, len(p2), 2):
                        work.append(p2[i:i + 2])
                    # uniform filler cadence: measured best across uniform /
                    # phase1-heavy (186us) / phase2-heavy (205us) variants
                    deferred = []
                    nf, fi = len(fillers), 0
                    prev_units = []
                    if g == 0 and nf:
                        # bridge the lead-in -> attention PE lull (the
                        # first scores wait on DVE qkT stores)
                        fillers[0]()
                        fi = 1

                    def retire(units):
                        # avs trail one step behind their sc/exp
                        for u in units:
                            emit_pair_av(u)
                        for u in units:
                            ln = u["ln"]
                            if u["bi"] + len(u["pair"]) == len(ln["ent"][3]):
                                norm_part1(ln)
                                deferred.append(ln)

                    for st, units in enumerate(work):
                        for ln in deferred:
                            norm_part2(ln)
                        deferred = []
                        for u in units:
                            blocks = u["ln"]["ent"][3]
                            u["pair"] = tuple(blocks[u["bi"]:u["bi"] + 2])
                            u["sc"] = sc_ps.tile([128, 1024], F32,
                                                 tag="sc2", name="sc2")
                        for half in range(2):
                            for u in units:
                                emit_pair_sc(u, half)
                        for u in units:
                            emit_pair_exp(u)
                        retire(prev_units)
                        prev_units = units
                        nsteps = len(work)
                        want = min(nf, ((st + 1) * nf + nsteps - 1) // nsteps)
                        while fi < want:
                            fillers[fi]()
                            fi += 1
                    # flush BEFORE the final retire: its av allocs may wait
                    # on these norms' reads (Tensor FIFO deadlock otherwise)
                    for ln in deferred:
                        norm_part2(ln)
                    deferred = []
                    retire(prev_units)
                    for ln in deferred:
                        norm_part2(ln)
                    while fi < nf:
                        fillers[fi]()
                        fi += 1

                lead_in()

                def flat(fls):
                    return [p for f in fls for p in f]

                grp_fillers = {
                    0: flat([qk_filler(ft, 1) for ft in range(4)]
                            + [v_filler(sb) for sb in range(4, 8)]),
                    1: flat([qk_filler(ft, 2) for ft in range(4)]
                            + [v_filler(sb) for sb in range(8, 12)]
                            + [op_filler(sb) for sb in range(0, 4)]),
                    2: flat([qk_filler(ft, 3) for ft in range(4)]
                            + [v_filler(sb) for sb in range(12, 16)]
                            + [op_filler(sb) for sb in range(4, 8)]),
                    3: flat([op_filler(sb) for sb in range(8, 12)]),
                }
                for g in range(4):
                    attn_group(g, grp_fillers[g])
                for p in flat([op_filler(sb) for sb in range(12, 16)]):
                    p()

    nc.compile()
    return nc


def make_core_inputs(c, x, W_packed, b_packed):
    """Host-side shard prep for core c (pure numpy reshuffles)."""
    k, b = c % 4, c // 4
    heads = [12 + k, 8 + k, 4 + k, k]          # slots A..D
    rows = np.concatenate([np.arange(h * 64, (h + 1) * 64) for h in heads])

    xT = np.ascontiguousarray(x[b].T)                       # [D, S]
    wq = W_packed[rows]                                     # [256, D]
    wk = W_packed[D + rows]
    wv = W_packed[2 * D + rows]
    wqkT = np.ascontiguousarray(np.concatenate([wq, wk], 0).T)  # [D, 512]
    wvT = np.ascontiguousarray(wv.T)                        # [D, 256]

    bq = b_packed[rows] / 8.0
    bk = b_packed[D + rows]
    bqk = np.stack([bq[:128], bq[128:], bk[:128], bk[128:]], 1)  # [128, 4]
    bqk = np.ascontiguousarray(bqk, dtype=np.float32)

    btab = np.zeros((128, TABW), np.float32)
    p = np.arange(128, dtype=np.float64)[:, None]
    scales = np.zeros(4, np.float64)
    for s in range(4):
        h = heads[s]
        slope = 2.0 ** (-(h + 1) * 8.0 / H)
        K, off0, tw, to = SLOT_KEEP[s], SLOT_OFF0[s], SLOT_TABW[s], SLOT_TABOFF[s]
        m = np.arange(tw, dtype=np.float64)[None, :]
        btab[:, to:to + tw] = (slope * (p + 128.0 * (m - (K - 1)) - off0)
                               ).astype(np.float32)
        # pre-round to bf16 so the scaled-v lanes and the denominator
        # ones-column use the IDENTICAL scale value (consistency beats
        # precision here: the factor cancels in softmax normalization)
        import ml_dtypes as _mld
        scales[s] = np.float64(np.float32(np.exp(-slope * 128.0))
                               .astype(_mld.bfloat16))
    import ml_dtypes
    ones = np.ones((65, 64), np.float32)
    vscr = np.broadcast_to(scales[None, :, None],
                           (128, 4, 64)).astype(np.float32)
    vsco = np.broadcast_to(scales[None, :, None],
                           (128, 4, 16)).astype(np.float32)
    return heads, {"xT": xT.astype(ml_dtypes.bfloat16),
                   "wqkT": wqkT.astype(ml_dtypes.bfloat16),
                   "wvT": wvT.astype(ml_dtypes.bfloat16),
                   "bqk": bqk, "btab": btab, "onesd": ones,
                   "vscr": np.ascontiguousarray(vscr),
                   "vsco": np.ascontiguousarray(vsco)}


_NC_CACHE = {}


def _get_program():
    if "nc" not in _NC_CACHE:
        _NC_CACHE["nc"] = build_program()
    return _NC_CACHE["nc"]


def kernel(x, W_packed, b_packed, W_out, b_out):
    x = np.asarray(x, np.float32)
    W_packed = np.asarray(W_packed, np.float32)
    b_packed = np.asarray(b_packed, np.float32)
    W_out = np.asarray(W_out, np.float32)
    b_out = np.asarray(b_out, np.float32)

    nc = _get_program()

    in_maps = []
    for c in range(NCORES):
        heads, m = make_core_inputs(c, x, W_packed, b_packed)
        cols = np.concatenate([np.arange(h * 64, (h + 1) * 64) for h in heads])
        import ml_dtypes
        m["woT"] = np.ascontiguousarray(W_out[:, cols].T).astype(
            ml_dtypes.bfloat16)
        in_maps.append(m)

    res = run_bass_kernel_spmd(nc, in_maps, core_ids=list(range(NCORES)))

    # Gather: sum partials per batch; add b_out and the folded v-bias term.
    b_v = b_packed[2 * D:]
    bias_row = (b_out + W_out @ b_v).astype(np.float32)     # [D]
    full = np.empty((B, S, D), np.float32)
    for b in range(B):
        acc = res.results[4 * b]["out"].astype(np.float32).copy()
        for c in range(4 * b + 1, 4 * b + 4):
            acc += res.results[c]["out"]
        full[b] = acc + bias_row
    return full


# revision 53
# speedup vs baseline: 1.1202x; 1.1202x over previous
"""Trainium2 Bass kernel for causal multi-head attention with ALiBi.

Computes, for x:[B,S,D]:
    qkv = x @ W_packed.T + b_packed ; q,k,v = split(qkv)
    heads -> scores = q k^T / sqrt(hd) + alibi_causal_bias
    out = softmax(scores) v -> merge heads -> out @ W_out.T + b_out

Sharding (8 cores): core c handles batch c//4 and heads {k, k+4, k+8, k+12}
(k = c%4), one head per "slot". Slot block-schedules are head-independent
(sized for the largest ALiBi window in the slot), so one SPMD program runs
on all 8 cores; only the data (weight slices, bias tables) differs.
Host sums the 4 out-projection partials per batch and adds
b_out + W_out @ b_v (the v-bias term commutes through attention).

ALiBi sparsity: head h attends effectively only ~24/slope_h positions back.
Slots keep only the causal k-blocks within that window (KEEP blocks).

Softmax without row-max: scores are O(+-6); exp is recentred per q-group by
a constant that cancels in normalization. In the transposed layout
scoresT[k,q] the ALiBi bias slope*(k - C_g) is per-partition so it rides
the Exp activation bias. Row sums come from a ones-row appended to v;
normalization divides by that row.

v3 pipeline:
- Lead-in computes only q/k columns 0-511 and v blocks 0-3 (m-outer, DMA
  overlapped); attention group 0 starts right after the x DMA lands. The
  remaining QKV work and the out-projection (one group behind) are woven
  between attention steps as PE filler bursts.
- Attention runs two lanes (slots A,B) of k-block PAIRS: each step does
  2 score MMs per lane into one 2-bank PSUM tile and ONE wide Exp over
  both blocks. The two blocks' ALiBi biases differ by slope*128; that
  offset is folded into pre-scaled copies of v (lanes 4-7 of v_t hold
  v*exp(-slope*128)), so one activation bias column serves the pair.
  Slots C and D follow as a single lane (C pairs, then D entries with
  both KEEP blocks merged the same way).
- A/B score MMs contract 64 rows on complementary partition halves and
  are emitted adjacently, so the PE runs them concurrently (row tiling).
"""

import os
import sys

import numpy as np

for _p in ("/opt/trn_rl_repo",):
    if os.path.isdir(_p) and _p not in sys.path:
        sys.path.append(_p)

import concourse.bacc as bacc
import concourse.bass as bass
import concourse.tile as tile
from concourse import mybir
from concourse.bass_utils import run_bass_kernel_spmd

B, S, D, H, HD = 2, 2048, 1024, 16, 64
NBLK = S // 128          # 16 k/q blocks
NCORES = 8

F32 = mybir.dt.float32
F32R = mybir.dt.float32r
BF16 = mybir.dt.bfloat16

# Slots A..D: per-core heads [12+k, 8+k, 4+k, k].  KEEP = causal k-blocks
# kept per q-block (window ~24/slope_h, slot max).  W = q-group width.
SLOT_KEEP = (17, 17, 6, 2)
SLOT_W = (512, 512, 512, 128)
SLOT_OFF0 = (128, 128, 128, 64)
SLOT_TABW = tuple(k + 3 if w == 512 else k for k, w in zip(SLOT_KEEP, SLOT_W))
SLOT_TABOFF = tuple(int(np.cumsum((0,) + SLOT_TABW)[i]) for i in range(4))
TABW = int(sum(SLOT_TABW))

# One Exp per k-block pair (True) vs per k-block (False). Merging halves
# ACT instructions via the pre-scaled v lanes; measured 9us faster than
# unmerged (174.4us vs 183.6us) at rel err 1.15e-2 vs 4.0e-3 — both well
# inside the 2e-2 gate, so the faster variant wins.
MERGED_EXP = True


def _slot_schedule(s):
    """Yield (g, q0, W, [(j, lo, tabcol, isdiag), ...]) per q-group."""
    K, W, _ = SLOT_KEEP[s], SLOT_W[s], SLOT_OFF0[s]
    out = []
    if W == 512:
        for g in range(S // 512):
            jlo = max(0, 4 * g + 3 - (K - 1))
            assert jlo % 2 == 0
            blocks = []
            for j in range(jlo, 4 * g + 4):
                lo = max(0, (j - 4 * g) * 128)
                m = j - 4 * g + (K - 1)
                blocks.append((j, lo, SLOT_TABOFF[s] + m, j >= 4 * g))
            assert len(blocks) % 2 == 0
            out.append((g, g * 512, 512, blocks))
    else:
        for i in range(NBLK):
            blocks = []
            for j in range(max(0, i - (K - 1)), i + 1):
                m = j - i + (K - 1)
                blocks.append((j, 0, SLOT_TABOFF[s] + m, j == i))
            out.append((i, i * 128, 128, blocks))
    return out


def build_program():
    nc = bacc.Bacc("TRN2", target_bir_lowering=False, debug=False,
                   num_devices=NCORES)

    xT = nc.dram_tensor("xT", [D, S], BF16, kind="ExternalInput")
    wqkT = nc.dram_tensor("wqkT", [D, 512], BF16, kind="ExternalInput")
    wvT = nc.dram_tensor("wvT", [D, 256], BF16, kind="ExternalInput")
    woT = nc.dram_tensor("woT", [256, D], BF16, kind="ExternalInput")
    bqk = nc.dram_tensor("bqk", [128, 4], F32, kind="ExternalInput")
    btab = nc.dram_tensor("btab", [128, TABW], F32, kind="ExternalInput")
    onesd = nc.dram_tensor("onesd", [65, 64], F32R, kind="ExternalInput")
    vscr = nc.dram_tensor("vscr", [128, 4, 64], F32, kind="ExternalInput")
    vsco = nc.dram_tensor("vsco", [128, 4, 16], F32, kind="ExternalInput")
    out = nc.dram_tensor("out", [S, D], BF16, kind="ExternalOutput")

    sched = [_slot_schedule(s) for s in range(4)]

    with tile.TileContext(nc) as tc:
        with tc.tile_pool(name="persist", bufs=1) as pp:
            qkT = [pp.tile([128, S], BF16, tag=f"qkT{t}", name=f"qkT{t}")
                   for t in range(4)]
            # v lanes 0-3: slot v (+ones col); lanes 4-7: v*exp(-slope*128)
            v_t = pp.tile([128, 8, NBLK, 65], BF16, tag="v", name="v")
            hoT = [pp.tile([128, S], BF16, tag=f"hoT{t}", name=f"hoT{t}")
                   for t in range(2)]
            btab_sb = pp.tile([128, TABW], F32, tag="btab", name="btab")
            bqk_sb = pp.tile([128, 4], F32, tag="bqk", name="bqk")
            ones_r = pp.tile([65, 64], F32R, tag="ones_r", name="ones_r")
            vscr_sb = pp.tile([128, 4, 64], F32, tag="vscr", name="vscr")
            vsco_sb = pp.tile([128, 4, 16, 1], F32, tag="vsco", name="vsco")
            xT_sb = [pp.tile([128, S], BF16, tag=f"x{m}", name=f"x{m}")
                     for m in range(8)]
            wqk_sb = [pp.tile([128, 512], BF16, tag=f"wqk{m}", name=f"wqk{m}")
                      for m in range(8)]
            wv_sb = [pp.tile([128, 256], BF16, tag=f"wv{m}", name=f"wv{m}")
                     for m in range(8)]
            wo_sb = [pp.tile([128, D], BF16, tag=f"wo{cc}", name=f"wo{cc}")
                     for cc in range(2)]

            # DMA priority: the 4MB x stream is the critical path — split
            # it across both queues; weights follow on the sync queue.
            for m in range(8):
                q = nc.sync if m % 2 == 0 else nc.gpsimd
                q.dma_start(xT_sb[m][:], xT[m * 128:(m + 1) * 128, :])
            for m in range(8):
                nc.sync.dma_start(wqk_sb[m][:],
                                  wqkT[m * 128:(m + 1) * 128, :])
            for m in range(8):
                nc.gpsimd.dma_start(wv_sb[m][:],
                                    wvT[m * 128:(m + 1) * 128, :])
            nc.gpsimd.dma_start(btab_sb[:], btab[:])
            nc.gpsimd.dma_start(bqk_sb[:], bqk[:])
            nc.gpsimd.dma_start(ones_r[:], onesd[:])
            nc.gpsimd.dma_start(vscr_sb[:], vscr[:])
            nc.gpsimd.dma_start(vsco_sb[:], vsco[:])
            # ones col for lanes 0-3; scaled ones col for lanes 4-7
            nc.gpsimd.memset(v_t[:, 0:4, :, 64:65], 1.0)
            nc.vector.tensor_copy(v_t[:, 4:8, :, 64:65], vsco_sb[:])
            for cc in range(2):
                nc.gpsimd.dma_start(wo_sb[cc][:],
                                    woT[cc * 128:(cc + 1) * 128, :])

            # PSUM: 8 banks = sc2 x2 (2 banks each) + av x2 + op x2.
            with (
                tc.tile_pool(name="etp", bufs=8) as etp,
                tc.tile_pool(name="nrm", bufs=3) as nrm,
                tc.tile_pool(name="obp", bufs=2) as obp,
                tc.tile_pool(name="ps_sc", bufs=2, space="PSUM") as sc_ps,
                tc.tile_pool(name="ps_av", bufs=2, space="PSUM") as av_ps,
                tc.tile_pool(name="ps_op", bufs=2, space="PSUM") as op_ps,
            ):
                def qk_store(ft, q4, ps):
                    scol = slice(q4 * 512, (q4 + 1) * 512)
                    # psum*scale + bias (1/sqrt(hd) folded into q)
                    nc.vector.tensor_scalar(
                        out=qkT[ft][:, scol], in0=ps[:],
                        scalar1=(0.125 if ft < 2 else 1.0),
                        scalar2=bqk_sb[:, ft:ft + 1],
                        op0=mybir.AluOpType.mult,
                        op1=mybir.AluOpType.add,
                    )

                def v_store(sb, ps):
                    re = ps[:].rearrange("p (s c) -> p s c", s=4)
                    nc.vector.tensor_copy(v_t[:, 0:4, sb, 0:64], re)
                    nc.vector.tensor_mul(v_t[:, 4:8, sb, 0:64], re,
                                         vscr_sb[:])

                def lead_in():
                    # q/k cols 0-511 + v blocks 0-3, m-outer (DMA-overlapped)
                    qk_ps = [sc_ps.tile([128, 1024], F32, tag="sc2",
                                        name=f"li_qk{i}") for i in range(2)]
                    v_ps = ([av_ps.tile([128, 256], F32, tag="av",
                                        name=f"li_v{i}") for i in range(2)]
                            + [op_ps.tile([128, 256], F32, tag="op",
                                          name=f"li_v{i + 2}")
                               for i in range(2)])
                    for m in range(8):
                        for ft in range(4):
                            nc.tensor.matmul(
                                qk_ps[ft // 2][:, (ft % 2) * 512:
                                               (ft % 2) * 512 + 512],
                                wqk_sb[m][:, ft * 128:(ft + 1) * 128],
                                xT_sb[m][:, 0:512],
                                start=(m == 0), stop=(m == 7),
                            )
                        for sb in range(4):
                            nc.tensor.matmul(
                                v_ps[sb][:],
                                xT_sb[m][:, sb * 128:(sb + 1) * 128],
                                wv_sb[m][:],
                                start=(m == 0), stop=(m == 7),
                            )
                    # kT (ft2) and qT (ft0) first: group-0 scores need them
                    for ft in (2, 0, 3, 1):
                        qk_store(ft, 0, qk_ps[ft // 2][:, (ft % 2) * 512:
                                                       (ft % 2) * 512 + 512])
                    for sb in range(4):
                        v_store(sb, v_ps[sb])

                # ---- fillers (PE bursts woven between attention steps) ----
                # each filler is split into two half-bursts so one burst
                # never adds a full 1.7us to the attention chain; pieces
                # stay adjacent in the list (at most one open accumulation,
                # which the op-tag rotation requires)
                def qk_filler(ft, q4):
                    st = {}

                    def p1():
                        st["ps"] = op_ps.tile([128, 512], F32, tag="op",
                                              name="qkf")
                        for m in range(4):
                            nc.tensor.matmul(
                                st["ps"][:],
                                wqk_sb[m][:, ft * 128:(ft + 1) * 128],
                                xT_sb[m][:, q4 * 512:(q4 + 1) * 512],
                                start=(m == 0), stop=False,
                            )

                    def p2():
                        for m in range(4, 8):
                            nc.tensor.matmul(
                                st["ps"][:],
                                wqk_sb[m][:, ft * 128:(ft + 1) * 128],
                                xT_sb[m][:, q4 * 512:(q4 + 1) * 512],
                                start=False, stop=(m == 7),
                            )
                        qk_store(ft, q4, st["ps"])
                    return [p1, p2]

                def v_filler(sb):
                    st = {}

                    def p1():
                        st["ps"] = op_ps.tile([128, 256], F32, tag="op",
                                              name="vf")
                        for m in range(4):
                            nc.tensor.matmul(
                                st["ps"][:],
                                xT_sb[m][:, sb * 128:(sb + 1) * 128],
                                wv_sb[m][:],
                                start=(m == 0), stop=False,
                            )

                    def p2():
                        for m in range(4, 8):
                            nc.tensor.matmul(
                                st["ps"][:],
                                xT_sb[m][:, sb * 128:(sb + 1) * 128],
                                wv_sb[m][:],
                                start=False, stop=(m == 7),
                            )
                        v_store(sb, st["ps"])
                    return [p1, p2]

                def op_filler(sb):
                    st = {}

                    def half(jh):
                        ps = op_ps.tile([128, 512], F32, tag="op",
                                        name="opps")
                        for cc in range(2):
                            nc.tensor.matmul(
                                ps[:],
                                hoT[cc][:, sb * 128:(sb + 1) * 128],
                                wo_sb[cc][:, jh * 512:(jh + 1) * 512],
                                start=(cc == 0), stop=(cc == 1),
                            )
                        nc.vector.tensor_copy(
                            st["ob"][:, jh * 512:(jh + 1) * 512], ps[:])

                    def p1():
                        st["ob"] = obp.tile([128, D], BF16, tag="ob",
                                            name="ob")
                        half(0)

                    def p2():
                        half(1)
                        nc.sync.dma_start(out[sb * 128:(sb + 1) * 128, :],
                                          st["ob"][:])
                    return [p1, p2]

                # ---- attention lanes ----
                def vlane(s, j, isdiag_member):
                    # merged-exp pairs use the 2nd member's bias col; the
                    # 1st member (even j for A/B/C, off-diag for D) rides
                    # the pre-scaled v lane instead.
                    if not MERGED_EXP:
                        return s
                    if s == 3:
                        return 3 if isdiag_member else 7
                    return (4 + s) if (j % 2 == 0) else s

                def emit_pair_sc(u, half):
                    ln = u["ln"]
                    s = ln["s"]
                    g, q0, W, blocks = ln["ent"]
                    pr = u["pair"]
                    if half >= len(pr):
                        return
                    j, lo, tcol, isdiag = pr[half]
                    po = (s % 2) * 64
                    qT_s = qkT[s // 2][po:po + 64, :]
                    kT_s = qkT[2 + s // 2][po:po + 64, :]
                    nc.tensor.matmul(
                        u["sc"][:, half * W + lo:half * W + W],
                        kT_s[:, j * 128:(j + 1) * 128],
                        qT_s[:, q0 + lo:q0 + W],
                    )

                def emit_pair_exp(u):
                    ln = u["ln"]
                    g, q0, W, blocks = ln["ent"]
                    pr = u["pair"]
                    lo0 = pr[0][1]
                    wd = W * len(pr)
                    tcol = pr[-1][2]
                    et = etp.tile([128, 1024], BF16, tag="et", name="et")
                    if MERGED_EXP:
                        nc.scalar.activation(
                            et[:, lo0:wd], u["sc"][:, lo0:wd],
                            mybir.ActivationFunctionType.Exp,
                            bias=btab_sb[:, tcol:tcol + 1], scale=1.0,
                        )
                    else:
                        for half, (j, lo, tc, isdiag) in enumerate(pr):
                            o = half * W
                            nc.scalar.activation(
                                et[:, o + lo:o + W], u["sc"][:, o + lo:o + W],
                                mybir.ActivationFunctionType.Exp,
                                bias=btab_sb[:, tc:tc + 1], scale=1.0,
                            )
                    for half, (j, lo, _tc, isdiag) in enumerate(pr):
                        if isdiag:
                            o = half * W + lo
                            # zero k>q inside the diagonal 128x128 block
                            nc.gpsimd.affine_select(
                                out=et[:, o:o + 128],
                                in_=et[:, o:o + 128],
                                compare_op=mybir.AluOpType.is_ge,
                                fill=0.0, base=0,
                                pattern=[[1, 128]],
                                channel_multiplier=-1,
                            )
                    u["et"] = et

                def emit_pair_av(u):
                    ln = u["ln"]
                    s = ln["s"]
                    g, q0, W, blocks = ln["ent"]
                    pr = u["pair"]
                    if ln["av"] is None:
                        ln["av"] = av_ps.tile([65, 512], F32, tag="av",
                                              name="av")
                    for half, (j, lo, _tc, isdiag) in enumerate(pr):
                        bi = u["bi"] + half
                        nc.tensor.matmul(
                            ln["av"][:, lo:W],
                            v_t[:, vlane(s, j, isdiag), j, :],
                            u["et"][:, half * W + lo:half * W + W],
                            start=(bi == 0),
                            stop=(bi == len(blocks) - 1),
                        )

                def norm_part1(ln):
                    g, q0, W, blocks = ln["ent"]
                    lr = nrm.tile([65, 512], F32R, tag="lr", name="lr")
                    nc.vector.tensor_copy(lr[64:65, :W],
                                          ln["av"][64:65, :W])
                    ln["lr"] = lr

                def norm_part2(ln):
                    s = ln["s"]
                    g, q0, W, blocks = ln["ent"]
                    po = (s % 2) * 64
                    hoT_s = hoT[s // 2]
                    av, lr = ln["av"], ln["lr"]
                    bps = op_ps.tile([64, 512], F32, tag="op", name="bps")
                    nc.tensor.matmul(
                        bps[:, :W], ones_r[64:65, 0:64], lr[64:65, :W])
                    binv = nrm.tile([64, 512], F32, tag="binv", name="binv")
                    nc.vector.reciprocal_approx_fast(out=binv[:, :W],
                                                     in_=bps[:, :W])
                    if po == 0:
                        nc.vector.tensor_mul(
                            hoT_s[0:64, q0:q0 + W], av[0:64, :W],
                            binv[:, :W])
                    else:
                        # DVE lanes can't shift partitions; bounce via DMA
                        tmp = nrm.tile([64, 512], BF16, tag="hotmp",
                                       name="hotmp")
                        nc.vector.tensor_mul(tmp[:, :W], av[0:64, :W],
                                             binv[:, :W])
                        nc.sync.dma_start(
                            hoT_s[64:128, q0:q0 + W], tmp[:, :W])

                def attn_group(g, fillers):
                    # phase 1: lanes A,B in k-block pairs; phase 2: C pairs
                    # then D entries, two pair-units per step, single chain.
                    def mklane(s, ent):
                        return {"s": s, "ent": ent, "av": None}

                    phase1 = [mklane(0, sched[0][g]), mklane(1, sched[1][g])]
                    phase2 = ([mklane(2, sched[2][g])]
                              + [mklane(3, sched[3][4 * g + i])
                                 for i in range(4)])
                    n1 = len(sched[0][g][3]) // 2
                    work = []
                    for st in range(n1):
                        work.append([{"ln": ln, "bi": st * 2}
                                     for ln in phase1])
                    p2 = []
                    for ln in phase2:
                        nb = len(ln["ent"][3])
                        p2.extend({"ln": ln, "bi": b}
                                  for b in range(0, nb, 2))
                    for i in range(0, len(p2), 2):
                        work.append(p2[i:i + 2])
                    # uniform filler cadence: measured best across uniform /
                    # phase1-heavy (186us) / phase2-heavy (205us) variants
                    deferred = []
                    nf, fi = len(fillers), 0
                    prev_units = []
                    if g == 0 and nf:
                        # bridge the lead-in -> attention PE lull (the
                        # first scores wait on DVE qkT stores)
                        fillers[0]()
                        fi = 1

                    def retire(units):
                        # avs trail one step behind their sc/exp
                        for u in units:
                            emit_pair_av(u)
                        for u in units:
                            ln = u["ln"]
                            if u["bi"] + len(u["pair"]) == len(ln["ent"][3]):
                                norm_part1(ln)
                                deferred.append(ln)

                    for st, units in enumerate(work):
                        for ln in deferred:
                            norm_part2(ln)
                        deferred = []
                        for u in units:
                            blocks = u["ln"]["ent"][3]
                            u["pair"] = tuple(blocks[u["bi"]:u["bi"] + 2])
                            u["sc"] = sc_ps.tile([128, 1024], F32,
                                                 tag="sc2", name="sc2")
                        for half in range(2):
                            for u in units:
                                emit_pair_sc(u, half)
                        for u in units:
                            emit_pair_exp(u)
                        retire(prev_units)
                        prev_units = units
                        nsteps = len(work)
                        want = min(nf, ((st + 1) * nf + nsteps - 1) // nsteps)
                        while fi < want:
                            fillers[fi]()
                            fi += 1
                    # flush BEFORE the final retire: its av allocs may wait
                    # on these norms' reads (Tensor FIFO deadlock otherwise)
                    for ln in deferred:
                        norm_part2(ln)
                    deferred = []
                    retire(prev_units)
                    for ln in deferred:
                        norm_part2(ln)
                    while fi < nf:
                        fillers[fi]()
                        fi += 1

                lead_in()

                def flat(fls):
                    return [p for f in fls for p in f]

                grp_fillers = {
                    0: flat([qk_filler(ft, 1) for ft in range(4)]
                            + [v_filler(sb) for sb in range(4, 8)]),
                    1: flat([qk_filler(ft, 2) for ft in range(4)]
                            + [v_filler(sb) for sb in range(8, 12)]
                            + [op_filler(sb) for sb in range(0, 4)]),
                    2: flat([qk_filler(ft, 3) for ft in range(4)]
                            + [v_filler(sb) for sb in range(12, 16)]
                            + [op_filler(sb) for sb in range(4, 8)]),
                    3: flat([op_filler(sb) for sb in range(8, 12)]),
                }
                for g in range(4):
                    attn_group(g, grp_fillers[g])
                for p in flat([op_filler(sb) for sb in range(12, 16)]):
                    p()

    nc.compile()
    return nc


def make_core_inputs(c, x, W_packed, b_packed):
    """Host-side shard prep for core c (pure numpy reshuffles)."""
    k, b = c % 4, c // 4
    heads = [12 + k, 8 + k, 4 + k, k]          # slots A..D
    rows = np.concatenate([np.arange(h * 64, (h + 1) * 64) for h in heads])

    xT = np.ascontiguousarray(x[b].T)                       # [D, S]
    wq = W_packed[rows]                                     # [256, D]
    wk = W_packed[D + rows]
    wv = W_packed[2 * D + rows]
    wqkT = np.ascontiguousarray(np.concatenate([wq, wk], 0).T)  # [D, 512]
    wvT = np.ascontiguousarray(wv.T)                        # [D, 256]

    bq = b_packed[rows] / 8.0
    bk = b_packed[D + rows]
    bqk = np.stack([bq[:128], bq[128:], bk[:128], bk[128:]], 1)  # [128, 4]
    bqk = np.ascontiguousarray(bqk, dtype=np.float32)

    btab = np.zeros((128, TABW), np.float32)
    p = np.arange(128, dtype=np.float64)[:, None]
    scales = np.zeros(4, np.float64)
    for s in range(4):
        h = heads[s]
        slope = 2.0 ** (-(h + 1) * 8.0 / H)
        K, off0, tw, to = SLOT_KEEP[s], SLOT_OFF0[s], SLOT_TABW[s], SLOT_TABOFF[s]
        m = np.arange(tw, dtype=np.float64)[None, :]
        btab[:, to:to + tw] = (slope * (p + 128.0 * (m - (K - 1)) - off0)
                               ).astype(np.float32)
        # pre-round to bf16 so the scaled-v lanes and the denominator
        # ones-column use the IDENTICAL scale value (consistency beats
        # precision here: the factor cancels in softmax normalization)
        import ml_dtypes as _mld
        scales[s] = np.float64(np.float32(np.exp(-slope * 128.0))
                               .astype(_mld.bfloat16))
    import ml_dtypes
    ones = np.ones((65, 64), np.float32)
    vscr = np.broadcast_to(scales[None, :, None],
                           (128, 4, 64)).astype(np.float32)
    vsco = np.broadcast_to(scales[None, :, None],
                           (128, 4, 16)).astype(np.float32)
    return heads, {"xT": xT.astype(ml_dtypes.bfloat16),
                   "wqkT": wqkT.astype(ml_dtypes.bfloat16),
                   "wvT": wvT.astype(ml_dtypes.bfloat16),
                   "bqk": bqk, "btab": btab, "onesd": ones,
                   "vscr": np.ascontiguousarray(vscr),
                   "vsco": np.ascontiguousarray(vsco)}


_NC_CACHE = {}


def _get_program():
    if "nc" not in _NC_CACHE:
        _NC_CACHE["nc"] = build_program()
    return _NC_CACHE["nc"]


def kernel(x, W_packed, b_packed, W_out, b_out):
    x = np.asarray(x, np.float32)
    W_packed = np.asarray(W_packed, np.float32)
    b_packed = np.asarray(b_packed, np.float32)
    W_out = np.asarray(W_out, np.float32)
    b_out = np.asarray(b_out, np.float32)

    nc = _get_program()

    in_maps = []
    for c in range(NCORES):
        heads, m = make_core_inputs(c, x, W_packed, b_packed)
        cols = np.concatenate([np.arange(h * 64, (h + 1) * 64) for h in heads])
        import ml_dtypes
        m["woT"] = np.ascontiguousarray(W_out[:, cols].T).astype(
            ml_dtypes.bfloat16)
        in_maps.append(m)

    res = run_bass_kernel_spmd(nc, in_maps, core_ids=list(range(NCORES)))

    # Gather: sum partials per batch; add b_out and the folded v-bias term.
    b_v = b_packed[2 * D:]
    bias_row = (b_out + W_out @ b_v).astype(np.float32)     # [D]
    full = np.empty((B, S, D), np.float32)
    for b in range(B):
        acc = res.results[4 * b]["out"].astype(np.float32).copy()
        for c in range(4 * b + 1, 4 * b + 4):
            acc += res.results[c]["out"]
        full[b] = acc + bias_row
    return full
